# revision 35
# baseline (speedup 1.0000x reference)
"""Per-channel Linear(seq->pred) over channels, 8-core channel-parallel Trainium2 kernel.

Math: y[b,p,c] = sum_s x[b,s,c] * W[c,p,s] + bias[c,p]

Strategy:
  - Shard channels C=321 across 8 cores (pad to 336 = 8*42, so each core
    works on 21 uniform channel pairs).
  - W is streamed as float8e3 (E3M4): host quantizes W*2^8 -> e3m4 and
    pre-scales x by 2^-8 in bf16 (powers of two, exact; PSUM accumulates
    the true fp32 y; measured rel err ~1.3e-2 < 2e-2 gate).
  - Contraction split into 6 K-chunks of 128 rows; global row 720
    carries the bias (x row = 2^-8, W row = bias*2^8). Chunk 5 only has
    81 real rows (640..720) and is loaded truncated; the stale SBUF /
    PE rows above are never contracted (K=81 matmuls).
  - Host pre-swizzles both inputs into the exact SBUF images so every
    DMA row is a long contiguous run and every full-chunk DMA spans all
    128 SBUF partitions (16-engine descriptor striping):
      wt[i, s, (k,c,p)] = W-pair i, K-chunk k row s      (fp8)
      xt[k, s, (c,b)]   = all-channel x, K-chunk k row s (bf16)
    Weight streams alternate between the two HWDGE queues (sync /
    scalar); W pair 0/1 are issued before the x chunks so the PE can
    start ~4us into the kernel.
  - Per channel pair and K-chunk, ONE 128x128 stationary load
    lhsT = [x_A | x_B] (fast-weight-load eligible), then 4 matmuls
    reuse it: W_A -> PSUM-A, W_B -> PSUM-B (N = 512 + 208, PSUM bank
    limit). Rows 64:128 of PSUM-A and 0:64 of PSUM-B are don't-care.
    The legalizer's redundant per-matmul LDWEIGHTS are deduped
    post-legalization (identical AP, no sync info) so one load serves
    all four matmuls and hides under the previous group's streams.
  - Result copied PSUM->SBUF as bf16 (DVE + ACT split) and DMA'd out.
"""

import numpy as np
import ml_dtypes

import concourse.bacc as bacc
import concourse.mybir as mybir
import concourse.tile as tile
from concourse.bass_utils import run_bass_kernel_spmd

F32 = mybir.dt.float32
BF16 = mybir.dt.bfloat16
F8E3 = mybir.dt.float8e3

B = 64          # batch
S = 720         # seq_len (contraction)
P = 720         # pred_len
C = 321         # channels
N_CORES = 8
CL = 41         # channels per core; 8*41 = 328 >= 321 (20 pairs + 1 single)
CPAD = N_CORES * CL
NPAIR = CL // 2
KCH = 128       # K-chunk rows
NKCH = 6        # chunks per channel
KLAST = S + 1 - 5 * KCH  # 86 real rows in the last chunk (incl. bias row)
SPAD = KCH * NKCH  # 768-row host image (720 data + bias + zeros)
NSPLIT = 512    # first matmul N (PSUM bank holds 512 f32)
WSCALE = 256.0  # W pre-scale (2^8), exactly undone by x pre-scale 2^-8

_CACHE: dict = {}


def _dedupe_ldweights(nc):
    """Remove back-to-back InstLdweights that reload identical weights.

    The legalizer emits one LDWEIGHTS per matmul; within a K-chunk our 4
    matmuls share one stationary, so 3 of 4 loads are redundant. Only
    drops loads with no sync waits/updates and an AP identical to the
    previously retained load, with nothing but matmuls in between.
    """
    removed = 0
    for blk in nc.m.functions[0].blocks:
        last_key = None
        new = []
        for inst in blk.instructions:
            if isinstance(inst, mybir.InstLdweights):
                key = (str(inst.ins[0]), str(inst.tile_position),
                       str(inst.perf_mode), str(inst.is_transpose))
                si = inst.sync_info
                clean = si is None or (not si.on_wait and not si.on_update)
                if clean and key == last_key:
                    removed += 1
                    continue
                last_key = key
            elif isinstance(inst, mybir.InstMatmult):
                pass  # matmuls don't disturb the loaded weights
            elif getattr(inst, "engine", None) == mybir.EngineType.PE:
                last_key = None  # any other PE op: be conservative
            new.append(inst)
        blk.instructions = new
    return removed


def _build_module():
    nc = bacc.Bacc("TRN2", target_bir_lowering=False, debug=False,
                   num_devices=N_CORES)
    # exact SBUF images, host-swizzled (long contiguous DMA rows)
    wt = nc.dram_tensor("wt", [NPAIR, KCH, 2 * NKCH, P], F8E3,
                        kind="ExternalInput").ap()
    ws = nc.dram_tensor("ws", [KCH, NKCH, P], F8E3,
                        kind="ExternalInput").ap()
    xt = nc.dram_tensor("xt", [NKCH, KCH, CL * B], BF16,
                        kind="ExternalInput").ap()
    y = nc.dram_tensor("y", [CL, B, P], BF16, kind="ExternalOutput").ap()

    def queue(j):  # alternate between the two HWDGE queues
        return nc.scalar if j % 2 else nc.sync

    with tile.TileContext(nc) as tc:
        with (
            tc.tile_pool(name="xp", bufs=1) as xp,
            tc.tile_pool(name="wp", bufs=6) as wp,
            tc.tile_pool(name="pa", bufs=2, space="PSUM") as pa,
            tc.tile_pool(name="pb", bufs=2, space="PSUM") as pb,
            tc.tile_pool(name="op", bufs=4) as op,
        ):
            xall = xp.tile([KCH, NKCH, CL, B], BF16, name="xall")
            wtiles = []

            def load_w(i):
                wbig = wp.tile([KCH, 2 * NKCH, P], F8E3, name=f"wbig{i}",
                               tag="wbig")
                # chunks 0..4 full height; chunk 5 truncated to 86 rows
                queue(i).dma_start(wbig[:, 0:2 * (NKCH - 1)],
                                   wt[i, :, 0:2 * (NKCH - 1)])
                queue(i).dma_start(wbig[0:KLAST, 2 * (NKCH - 1):],
                                   wt[i, 0:KLAST, 2 * (NKCH - 1):])
                wtiles.append(wbig)

            def load_x(k, cl, ch, q):
                rows = KCH if k < NKCH - 1 else KLAST
                q.dma_start(
                    xall[0:rows, k, cl:ch],
                    xt[k, 0:rows].rearrange("s (c b) -> s c b",
                                            c=CL)[:, cl:ch])

            # W pair 0 on the sync queue while the x chunk heads (first 6
            # channels -> pairs 0-2) stream on the other queue, so the PE
            # starts as soon as W0 + head 0 land; the x tails follow after
            # W1/W2 so they don't delay the early weight streams.
            load_w(0)
            for k in range(NKCH):
                load_x(k, 0, 10, queue(k + 1))

            prev_mm = None
            for i in range(NPAIR):
                if i >= 1:
                    load_w(i)
                if i == 1:
                    for k in (0, 2, 4):
                        load_x(k, 10, CL, nc.scalar)
                elif i == 2:
                    for k in (1, 3, 5):
                        load_x(k, 10, CL, nc.sync)
                wbig = wtiles[i]
                c0 = 2 * i
                psA = pa.tile([2 * B, P], F32, name=f"psA{i}", tag="psA")
                psB = pb.tile([2 * B, P], F32, name=f"psB{i}", tag="psB")
                for k in range(NKCH):
                    st, sp = (k == 0), (k == NKCH - 1)
                    rows = KCH if k < NKCH - 1 else KLAST
                    # one full-width stationary: cols 0:64 = ch A, 64:128 = B
                    lhsT = xall[0:rows, k, c0:c0 + 2]
                    mms = [
                        nc.tensor.matmul(psA[:, 0:NSPLIT], lhsT,
                                         wbig[0:rows, 2 * k, 0:NSPLIT],
                                         start=st, stop=sp),
                        nc.tensor.matmul(psA[:, NSPLIT:P], lhsT,
                                         wbig[0:rows, 2 * k, NSPLIT:P],
                                         start=st, stop=sp),
                        nc.tensor.matmul(psB[:, 0:NSPLIT], lhsT,
                                         wbig[0:rows, 2 * k + 1, 0:NSPLIT],
                                         start=st, stop=sp),
                        nc.tensor.matmul(psB[:, NSPLIT:P], lhsT,
                                         wbig[0:rows, 2 * k + 1, NSPLIT:P],
                                         start=st, stop=sp),
                    ]
                    # chain nosync deps so the scheduler can't reorder any
                    # matmul across the (deduped) weight loads
                    for mm in mms:
                        if prev_mm is not None:
                            mm.ins.add_dependency(
                                prev_mm.ins.name,
                                mybir.DependencyInfo.NO_SYNC_ONLY)
                        prev_mm = mm
                # evictions can cross PSUM banks (only matmul writes can't);
                # split 448/272 to balance DVE vs ACT throughput
                ev = 448
                out = op.tile([2 * B, P], BF16, name=f"out{i}", tag="out")
                nc.vector.tensor_copy(out[0:B, 0:ev], psA[0:B, 0:ev])
                nc.vector.tensor_copy(out[B:2 * B, 0:ev],
                                      psB[B:2 * B, 0:ev])
                nc.scalar.copy(out[0:B, ev:P], psA[0:B, ev:P])
                nc.scalar.copy(out[B:2 * B, ev:P],
                               psB[B:2 * B, ev:P])
                queue(i).dma_start(
                    y[c0:c0 + 2].rearrange("c b p -> (c b) p"), out[:])

            # 41st channel: structurally a pair (full 128-col stationary
            # [x_39|x_40], uniform tile shapes, no PE tile-mode switch) but
            # streaming only W_40: rows 0:64 of PSUM are don't-care, rows
            # 64:128 are y_40. Only half the usual stream => the 2.4%
            # padding the 42-channel layout wasted is gone.
            wsm = wp.tile([KCH, 2 * NKCH, P], F8E3, name="wsingle",
                          tag="wbig")
            nc.sync.dma_start(wsm[:, 0:NKCH - 1], ws[:, 0:NKCH - 1])
            nc.sync.dma_start(wsm[0:KLAST, NKCH - 1:NKCH],
                              ws[0:KLAST, NKCH - 1:])
            psS = pa.tile([2 * B, P], F32, name="psS", tag="psA")
            for k in range(NKCH):
                st, sp = (k == 0), (k == NKCH - 1)
                rows = KCH if k < NKCH - 1 else KLAST
                lhsT = xall[0:rows, k, CL - 2:CL]
                mms = [
                    nc.tensor.matmul(psS[:, 0:NSPLIT], lhsT,
                                     wsm[0:rows, k, 0:NSPLIT],
                                     start=st, stop=sp),
                    nc.tensor.matmul(psS[:, NSPLIT:P], lhsT,
                                     wsm[0:rows, k, NSPLIT:P],
                                     start=st, stop=sp),
                ]
                for mm in mms:
                    mm.ins.add_dependency(prev_mm.ins.name,
                                          mybir.DependencyInfo.NO_SYNC_ONLY)
                    prev_mm = mm
            outS = op.tile([2 * B, P], BF16, name="outS", tag="out")
            nc.vector.tensor_copy(outS[B:2 * B, 0:448], psS[B:2 * B, 0:448])
            nc.scalar.copy(outS[B:2 * B, 448:P], psS[B:2 * B, 448:P])
            nc.scalar.dma_start(y[CL - 1], outS[B:2 * B])

    n = _dedupe_ldweights(nc)
    assert n >= NPAIR * NKCH * 3, f"deduped {n} ldweights"
    nc.compile()
    return nc


def _get_module():
    if "nc" not in _CACHE:
        _CACHE["nc"] = _build_module()
    return _CACHE["nc"]


def _prep_inputs(x, W, b):
    x = np.asarray(x, dtype=np.float32)
    W = np.asarray(W, dtype=np.float32)
    b = np.asarray(b, dtype=np.float32)
    wt = np.zeros((CPAD, SPAD, P), dtype=np.float32)
    wt[:C, :S, :] = W.transpose(0, 2, 1) * WSCALE
    wt[:C, S, :] = b * WSCALE
    wt8 = wt.astype(ml_dtypes.float8_e3m4)

    xt = np.zeros((SPAD, CPAD, B), dtype=np.float32)
    xt[:S, :C, :] = x.transpose(1, 2, 0) * (1.0 / WSCALE)
    xt[S, :C, :] = 1.0 / WSCALE
    xt16 = np.ascontiguousarray(
        xt.astype(ml_dtypes.bfloat16).reshape(NKCH, KCH, CPAD, B))

    in_maps = []
    for i in range(N_CORES):
        wc = wt8[i * CL:(i + 1) * CL]  # 41 channels for this core
        # pairs: swizzle to [pair, s, (k, c, p)]
        wp_ = np.ascontiguousarray(
            wc[:2 * NPAIR].reshape(NPAIR, 2, NKCH, KCH, P)
            .transpose(0, 3, 2, 1, 4)).reshape(NPAIR, KCH, 2 * NKCH, P)
        # single 41st channel: [s, k, p]
        ws_ = np.ascontiguousarray(
            wc[2 * NPAIR].reshape(NKCH, KCH, P).transpose(1, 0, 2))
        in_maps.append({
            "wt": wp_,
            "ws": ws_,
            "xt": np.ascontiguousarray(
                xt16[:, :, i * CL:(i + 1) * CL, :]).reshape(
                    NKCH, KCH, CL * B),
        })
    return in_maps


def _gather(results):
    ys = np.concatenate([results[i]["y"] for i in range(N_CORES)], axis=0)
    return np.ascontiguousarray(
        ys[:C].astype(np.float32).transpose(1, 2, 0))


def run(x, W, b, **run_kwargs):
    """Full pipeline, returns (output, BassKernelResults)."""
    nc = _get_module()
    in_maps = _prep_inputs(x, W, b)
    res = run_bass_kernel_spmd(nc, in_maps, list(range(N_CORES)), **run_kwargs)
    return _gather(res.results), res


def kernel(x, W, b):
    out, _ = run(x, W, b)
    return out


# revision 36
# speedup vs baseline: 1.1060x; 1.1060x over previous
"""Per-channel Linear(seq->pred) over channels, 8-core channel-parallel Trainium2 kernel.

Math: y[b,p,c] = sum_s x[b,s,c] * W[c,p,s] + bias[c,p]

Strategy:
  - Shard channels C=321 across 8 cores (pad to 336 = 8*42, so each core
    works on 21 uniform channel pairs).
  - W is streamed as float8e3 (E3M4): host quantizes W*2^8 -> e3m4 and
    pre-scales x by 2^-8 in bf16 (powers of two, exact; PSUM accumulates
    the true fp32 y; measured rel err ~1.3e-2 < 2e-2 gate).
  - Contraction split into 6 K-chunks of 128 rows; global row 720
    carries the bias (x row = 2^-8, W row = bias*2^8). Chunk 5 only has
    81 real rows (640..720) and is loaded truncated; the stale SBUF /
    PE rows above are never contracted (K=81 matmuls).
  - Host pre-swizzles both inputs into the exact SBUF images so every
    DMA row is a long contiguous run and every full-chunk DMA spans all
    128 SBUF partitions (16-engine descriptor striping):
      wt[i, s, (k,c,p)] = W-pair i, K-chunk k row s      (fp8)
      xt[k, s, (c,b)]   = all-channel x, K-chunk k row s (bf16)
    Weight streams alternate between the two HWDGE queues (sync /
    scalar); W pair 0/1 are issued before the x chunks so the PE can
    start ~4us into the kernel.
  - Per channel pair and K-chunk, ONE 128x128 stationary load
    lhsT = [x_A | x_B] (fast-weight-load eligible), then 4 matmuls
    reuse it: W_A -> PSUM-A, W_B -> PSUM-B (N = 512 + 208, PSUM bank
    limit). Rows 64:128 of PSUM-A and 0:64 of PSUM-B are don't-care.
    The legalizer's redundant per-matmul LDWEIGHTS are deduped
    post-legalization (identical AP, no sync info) so one load serves
    all four matmuls and hides under the previous group's streams.
  - Result copied PSUM->SBUF as bf16 (DVE + ACT split) and DMA'd out.
"""

import numpy as np
import ml_dtypes

import concourse.bacc as bacc
import concourse.mybir as mybir
import concourse.tile as tile
from concourse.bass_utils import run_bass_kernel_spmd

F32 = mybir.dt.float32
BF16 = mybir.dt.bfloat16
F8E3 = mybir.dt.float8e3

B = 64          # batch
S = 720         # seq_len (contraction)
P = 720         # pred_len
C = 321         # channels
N_CORES = 8
CL = 42         # channels per core; 8*42 = 336 >= 321
CPAD = N_CORES * CL
NPAIR = CL // 2
KCH = 128       # K-chunk rows
NKCH = 6        # chunks per channel
KLAST = S + 1 - 5 * KCH  # 86 real rows in the last chunk (incl. bias row)
SPAD = KCH * NKCH  # 768-row host image (720 data + bias + zeros)
NSPLIT = 512    # first matmul N (PSUM bank holds 512 f32)
WSCALE = 256.0  # W pre-scale (2^8), exactly undone by x pre-scale 2^-8

_CACHE: dict = {}


def _dedupe_ldweights(nc):
    """Remove back-to-back InstLdweights that reload identical weights.

    The legalizer emits one LDWEIGHTS per matmul; within a K-chunk our 4
    matmuls share one stationary, so 3 of 4 loads are redundant. Only
    drops loads with no sync waits/updates and an AP identical to the
    previously retained load, with nothing but matmuls in between.
    """
    removed = 0
    for blk in nc.m.functions[0].blocks:
        last_key = None
        new = []
        for inst in blk.instructions:
            if isinstance(inst, mybir.InstLdweights):
                key = (str(inst.ins[0]), str(inst.tile_position),
                       str(inst.perf_mode), str(inst.is_transpose))
                si = inst.sync_info
                clean = si is None or (not si.on_wait and not si.on_update)
                if clean and key == last_key:
                    removed += 1
                    continue
                last_key = key
            elif isinstance(inst, mybir.InstMatmult):
                pass  # matmuls don't disturb the loaded weights
            elif getattr(inst, "engine", None) == mybir.EngineType.PE:
                last_key = None  # any other PE op: be conservative
            new.append(inst)
        blk.instructions = new
    return removed


def _build_module():
    nc = bacc.Bacc("TRN2", target_bir_lowering=False, debug=False,
                   num_devices=N_CORES)
    # exact SBUF images, host-swizzled (long contiguous DMA rows)
    wt = nc.dram_tensor("wt", [NPAIR, KCH, 2 * NKCH, P], F8E3,
                        kind="ExternalInput").ap()
    xt = nc.dram_tensor("xt", [NKCH, KCH, CL * B], BF16,
                        kind="ExternalInput").ap()
    y = nc.dram_tensor("y", [CL, B, P], BF16, kind="ExternalOutput").ap()

    def queue(j):  # alternate between the two HWDGE queues
        return nc.scalar if j % 2 else nc.sync

    with tile.TileContext(nc) as tc:
        with (
            tc.tile_pool(name="xp", bufs=1) as xp,
            tc.tile_pool(name="wp", bufs=6) as wp,
            tc.tile_pool(name="pa", bufs=2, space="PSUM") as pa,
            tc.tile_pool(name="pb", bufs=2, space="PSUM") as pb,
            tc.tile_pool(name="op", bufs=4) as op,
        ):
            xall = xp.tile([KCH, NKCH, CL, B], BF16, name="xall")
            wtiles = []

            def load_w(i):
                wbig = wp.tile([KCH, 2 * NKCH, P], F8E3, name=f"wbig{i}",
                               tag="wbig")
                # chunks 0..4 full height; chunk 5 truncated to 86 rows
                queue(i).dma_start(wbig[:, 0:2 * (NKCH - 1)],
                                   wt[i, :, 0:2 * (NKCH - 1)])
                queue(i).dma_start(wbig[0:KLAST, 2 * (NKCH - 1):],
                                   wt[i, 0:KLAST, 2 * (NKCH - 1):])
                wtiles.append(wbig)

            def load_x(k, cl, ch, q):
                rows = KCH if k < NKCH - 1 else KLAST
                q.dma_start(
                    xall[0:rows, k, cl:ch],
                    xt[k, 0:rows].rearrange("s (c b) -> s c b",
                                            c=CL)[:, cl:ch])

            # W pair 0 on the sync queue while the x chunk heads (first 6
            # channels -> pairs 0-2) stream on the other queue, so the PE
            # starts as soon as W0 + head 0 land; the x tails follow after
            # W1/W2 so they don't delay the early weight streams.
            load_w(0)
            for k in range(NKCH):
                load_x(k, 0, 10, queue(k + 1))

            prev_mm = None
            for i in range(NPAIR):
                if i >= 1:
                    load_w(i)
                if i == 1:
                    for k in (0, 2, 4):
                        load_x(k, 10, CL, nc.scalar)
                elif i == 2:
                    for k in (1, 3, 5):
                        load_x(k, 10, CL, nc.sync)
                wbig = wtiles[i]
                c0 = 2 * i
                psA = pa.tile([2 * B, P], F32, name=f"psA{i}", tag="psA")
                psB = pb.tile([2 * B, P], F32, name=f"psB{i}", tag="psB")
                for k in range(NKCH):
                    st, sp = (k == 0), (k == NKCH - 1)
                    rows = KCH if k < NKCH - 1 else KLAST
                    # one full-width stationary: cols 0:64 = ch A, 64:128 = B
                    lhsT = xall[0:rows, k, c0:c0 + 2]
                    mms = [
                        nc.tensor.matmul(psA[:, 0:NSPLIT], lhsT,
                                         wbig[0:rows, 2 * k, 0:NSPLIT],
                                         start=st, stop=sp),
                        nc.tensor.matmul(psA[:, NSPLIT:P], lhsT,
                                         wbig[0:rows, 2 * k, NSPLIT:P],
                                         start=st, stop=sp),
                        nc.tensor.matmul(psB[:, 0:NSPLIT], lhsT,
                                         wbig[0:rows, 2 * k + 1, 0:NSPLIT],
                                         start=st, stop=sp),
                        nc.tensor.matmul(psB[:, NSPLIT:P], lhsT,
                                         wbig[0:rows, 2 * k + 1, NSPLIT:P],
                                         start=st, stop=sp),
                    ]
                    # chain nosync deps so the scheduler can't reorder any
                    # matmul across the (deduped) weight loads
                    for mm in mms:
                        if prev_mm is not None:
                            mm.ins.add_dependency(
                                prev_mm.ins.name,
                                mybir.DependencyInfo.NO_SYNC_ONLY)
                        prev_mm = mm
                # evictions can cross PSUM banks (only matmul writes can't);
                # split 448/272 to balance DVE vs ACT throughput
                ev = 448
                out = op.tile([2 * B, P], BF16, name=f"out{i}", tag="out")
                nc.vector.tensor_copy(out[0:B, 0:ev], psA[0:B, 0:ev])
                nc.vector.tensor_copy(out[B:2 * B, 0:ev],
                                      psB[B:2 * B, 0:ev])
                nc.scalar.copy(out[0:B, ev:P], psA[0:B, ev:P])
                nc.scalar.copy(out[B:2 * B, ev:P],
                               psB[B:2 * B, ev:P])
                queue(i).dma_start(
                    y[c0:c0 + 2].rearrange("c b p -> (c b) p"), out[:])

    n = _dedupe_ldweights(nc)
    assert n == NPAIR * NKCH * 3, f"deduped {n} ldweights"
    nc.compile()
    return nc


def _get_module():
    if "nc" not in _CACHE:
        _CACHE["nc"] = _build_module()
    return _CACHE["nc"]


def _prep_inputs(x, W, b):
    x = np.asarray(x, dtype=np.float32)
    W = np.asarray(W, dtype=np.float32)
    b = np.asarray(b, dtype=np.float32)
    wt = np.zeros((CPAD, SPAD, P), dtype=np.float32)
    wt[:C, :S, :] = W.transpose(0, 2, 1) * WSCALE
    wt[:C, S, :] = b * WSCALE
    wt8 = wt.astype(ml_dtypes.float8_e3m4)
    # swizzle to the SBUF image: [pair, s, (k, c, p)]
    wt8 = np.ascontiguousarray(
        wt8.reshape(CPAD // 2, 2, NKCH, KCH, P)
        .transpose(0, 3, 2, 1, 4)
        .reshape(CPAD // 2, KCH, 2 * NKCH * P))

    xt = np.zeros((SPAD, CPAD, B), dtype=np.float32)
    xt[:S, :C, :] = x.transpose(1, 2, 0) * (1.0 / WSCALE)
    xt[S, :C, :] = 1.0 / WSCALE
    xt16 = np.ascontiguousarray(
        xt.astype(ml_dtypes.bfloat16).reshape(NKCH, KCH, CPAD, B))

    in_maps = []
    for i in range(N_CORES):
        in_maps.append({
            "wt": np.ascontiguousarray(
                wt8[i * NPAIR:(i + 1) * NPAIR]).reshape(
                    NPAIR, KCH, 2 * NKCH, P),
            "xt": np.ascontiguousarray(
                xt16[:, :, i * CL:(i + 1) * CL, :]).reshape(
                    NKCH, KCH, CL * B),
        })
    return in_maps


def _gather(results):
    ys = np.concatenate([results[i]["y"] for i in range(N_CORES)], axis=0)
    return np.ascontiguousarray(
        ys[:C].astype(np.float32).transpose(1, 2, 0))


def run(x, W, b, **run_kwargs):
    """Full pipeline, returns (output, BassKernelResults)."""
    nc = _get_module()
    in_maps = _prep_inputs(x, W, b)
    res = run_bass_kernel_spmd(nc, in_maps, list(range(N_CORES)), **run_kwargs)
    return _gather(res.results), res


def kernel(x, W, b):
    out, _ = run(x, W, b)
    return out
